# revision 18
# baseline (speedup 1.0000x reference)
"""Trainium2 Bass kernel for nn_AutoeclecticResponderHead.

Math (per row b):
    w      = softmax(se_b * gate_w + gate_b)          # [4]
    mix    = sigmoid(curv_b)
    out_b  = (1-mix) * (state_b @ prj_w + prj_b) + mix * sum_m w_m * (state_b @ W_m)
           = sum_{k=0..4} c_k[b] * (state_b @ A_k)  +  c_4[b] * prj_b
    with A_0..3 = modulation_basis modes (c_k = mix*w_k), A_4 = prj_w (c_4 = 1-mix).

Sharding: data-parallel over batch, 1024 rows per core, weights replicated.

Per-core kernel (v5):
  - Host pre-casts state + weights to bf16 and pre-transposes layouts, so the
    device does no dtype conversion and input DMA bytes are halved vs fp32.
  - 640 bf16 matmuls: stationary state tile [128,128], moving weight piece
    [128,512] (dedicated contiguous tiles - a strided slice of a larger tile
    measurably slows the PE, and contiguous pieces DMA at full line rate).
  - Weight chunks are streamed just-in-time: chunk (o,k+1) DMAs are emitted
    between chunk (o,k)'s matmul groups and throttled by weight-pool buffer
    reuse, which pins the scheduler to the consumption order (an up-front
    DMA flood gets reordered and starves early chunks).
  - Each (b, o-half) output block drains on the otherwise-idle sync ring
    right after its last combine, so the tail is one 256KB drain instead of
    the baseline's 11us bunched write-out.
"""

import os
import numpy as np
import ml_dtypes

B, H, O, M = 8192, 1024, 1024, 4
NCORES = 8
BL = B // NCORES          # rows per core
NB = BL // 128            # b tiles per core
NH = H // 128             # h (contraction) tiles
NO = O // 512             # output column tiles
NK = M + 1                # modes + base projection

_cached_nc = None
LAST_EXEC_TIME_NS = None
LAST_TRACE = None


def _build_nc():
    import concourse.bacc as bacc
    import concourse.tile as tile
    from concourse import mybir

    f32 = mybir.dt.float32
    bf16 = mybir.dt.bfloat16
    f8 = mybir.dt.float8e4
    DR = mybir.MatmulPerfMode.DoubleRow
    Alu = mybir.AluOpType
    Act = mybir.ActivationFunctionType
    AxX = mybir.AxisListType.X

    nc = bacc.Bacc("TRN2", target_bir_lowering=False, debug=False,
                   num_devices=NCORES)

    # lhsT layout: [b_tile, h_in(part), h_tile, row]
    stateT = nc.dram_tensor("stateT", [NB, 128, NH, 128], bf16,
                            kind="ExternalInput").ap()
    stateT8 = nc.dram_tensor("stateT8", [NB, 128, NH, 128], f8,
                             kind="ExternalInput").ap()
    # bf16 moving chunks (modes 2,3): [j, o, 128(part), h, 512]
    wts = nc.dram_tensor("wts", [M - 2, NO, 128, NH, 512], bf16,
                         kind="ExternalInput").ap()
    m08 = nc.dram_tensor("m08", [NO, 128, NH, 512], f8,
                         kind="ExternalInput").ap()
    m18 = nc.dram_tensor("m18", [NO, 128, NH, 512], f8,
                         kind="ExternalInput").ap()
    prj8 = nc.dram_tensor("prj8", [NO, 128, NH, 512], f8,
                          kind="ExternalInput").ap()
    sc = nc.dram_tensor("sc", [128, 2 * NB], f32, kind="ExternalInput").ap()
    gwb = nc.dram_tensor("gwb", [128, 2 * M], f32, kind="ExternalInput").ap()
    pb = nc.dram_tensor("pb", [128, O], f32, kind="ExternalInput").ap()
    out = nc.dram_tensor("out", [BL, O], f32, kind="ExternalOutput").ap()

    out_r = out.rearrange("(t p) o -> p t o", p=128)            # [128, NB, O]

    with tile.TileContext(nc) as tc:
        with (
            tc.tile_pool(name="big", bufs=1) as bigpool,
            tc.tile_pool(name="w", bufs=3) as wpool,
            tc.tile_pool(name="w0", bufs=1) as w0pool,
            tc.tile_pool(name="st", bufs=NB) as stpool,
            tc.tile_pool(name="st8", bufs=NB) as st8pool,
            tc.tile_pool(name="p8", bufs=3 * NO) as p8pool,
            tc.tile_pool(name="acc", bufs=NB) as apool,
            tc.tile_pool(name="g", bufs=NB) as gpool,
            tc.tile_pool(name="c", bufs=NB) as cpool,
            tc.tile_pool(name="ps", bufs=8, space="PSUM") as ppool,
        ):
            # PE warm-up first on the vector queue (no DMA dependency):
            # ramps the HAM clock during the initial DMA window.
            warm_in = bigpool.tile([128, 512], bf16, tag="warm")
            nc.vector.memset(warm_in[:], 0.0)
            warm_ps = ppool.tile([128, 512], f32, tag="ps")
            for i in range(16):
                nc.tensor.matmul(
                    warm_ps[:], lhsT=warm_in[:, 0:128], rhs=warm_in[:],
                    start=(i == 0), stop=(i == 15))

            # DMA plan. dma_start issue costs ~1.3us of queue time, so the
            # latency-critical queues carry few, big transfers, and the
            # scalar queue runs the gating ACT ops BEFORE its weight DMAs.
            #   gpsimd: sc, gwb (gating inputs), m18[0], s1 chunk, pb,
            #           o1 chunks + o1 f8 weights
            #   sync:   st8 x8 (f8 phases open the kernel), stb x8, prj8[0]
            #   scalar: m08[0], s0 chunk split in two halves
            sc_t = bigpool.tile([128, 2 * NB], f32, tag="sc")
            nc.gpsimd.dma_start(sc_t[:], sc[:])
            gwb_t = bigpool.tile([128, 2 * M], f32, tag="gwb")
            nc.gpsimd.dma_start(gwb_t[:], gwb[:])
            m18_t = [None] * NO
            m18_t[0] = p8pool.tile([128, NH, 512], f8, name="m18_0", tag="p8")
            nc.gpsimd.dma_start(m18_t[0][:], m18[0])

            st8 = []
            for b in range(NB):
                s8 = st8pool.tile([128, NH, 128], f8, tag="st8")
                nc.sync.dma_start(s8[:], stateT8[b])
                st8.append(s8)

            m08_t = [None] * NO
            m08_t[0] = p8pool.tile([128, NH, 512], f8, name="m08_0", tag="p8")
            nc.scalar.dma_start(m08_t[0][:], m08[0])
            # mode2 o0 chunk in two halves for an earlier first matmul
            s0a = w0pool.tile([128, NH // 2, 512], bf16, tag="w0a")
            nc.scalar.dma_start(s0a[:], wts[0][0][:, 0:NH // 2, :])
            s0b = w0pool.tile([128, NH // 2, 512], bf16, tag="w0b")
            nc.scalar.dma_start(s0b[:], wts[0][0][:, NH // 2:NH, :])

            # Gating (vector + scalar ACT, emitted before any further
            # dma_start lands on the scalar queue). ctile columns:
            # [0:M] = mix*softmax, [M] = (1-mix), [M+1] = (1-mix)/32.
            logits, nmxs, es, mixs, ctiles = [], [], [], [], []
            for j in range(NB):
                s = sc_t[:, j:j + 1]
                logit = gpool.tile([128, M], f32, tag="logit")
                nc.vector.scalar_tensor_tensor(
                    logit[:], gwb_t[:, 0:M], s, gwb_t[:, M:2 * M],
                    Alu.mult, Alu.add)
                logits.append(logit)
                nmx = gpool.tile([128, 1], f32, tag="nmx")
                nc.vector.tensor_reduce(
                    nmx[:], logit[:], axis=AxX, op=Alu.max, negate=True)
                nmxs.append(nmx)
            for j in range(NB):
                e = gpool.tile([128, M], f32, tag="e")
                nc.scalar.activation(e[:], logits[j][:], Act.Exp, bias=nmxs[j][:])
                es.append(e)
            for j in range(NB):
                mix = gpool.tile([128, 1], f32, tag="mix")
                nc.scalar.activation(
                    mix[:], sc_t[:, NB + j:NB + j + 1], Act.Sigmoid)
                mixs.append(mix)
            for j in range(NB):
                sm = gpool.tile([128, 1], f32, tag="sm")
                nc.vector.reduce_sum(sm[:], es[j][:], axis=AxX)
                rin = gpool.tile([128, 1], f32, tag="rin")
                nc.vector.reciprocal(rin[:], sm[:])
                c = cpool.tile([128, M + 2], f32, tag="c")
                nc.vector.tensor_scalar(
                    c[:, 0:M], es[j][:], rin[:], mixs[j][:], Alu.mult, Alu.mult)
                nc.vector.tensor_scalar(
                    c[:, M:M + 1], mixs[j][:], -1.0, 1.0, Alu.mult, Alu.add)
                nc.vector.tensor_scalar(
                    c[:, M + 1:M + 2], mixs[j][:], -1.0 / 32.0, 1.0 / 32.0,
                    Alu.mult, Alu.add)
                ctiles.append(c)

            # Remaining inputs, after the gating chain is unblocked.
            stb = []
            for b in range(NB):
                st = stpool.tile([128, NH, 128], bf16, tag="st")
                # last two tiles ride the scalar ring (free after its three
                # early loads) for arrival margin in the late bf16 groups
                ring = nc.sync if b < NB - 2 else nc.scalar
                ring.dma_start(st[:], stateT[b])
                stb.append(st)
            prj8_t = [None] * NO
            prj8_t[0] = p8pool.tile([128, NH, 512], f8, name="prj8_0", tag="p8")
            nc.sync.dma_start(prj8_t[0][:], prj8[0])

            s1 = wpool.tile([128, NH, 512], bf16, name="s1", tag="w")
            nc.gpsimd.dma_start(s1[:], wts[1][0])
            pb_t = bigpool.tile([128, O], f32, tag="pb")
            nc.gpsimd.dma_start(pb_t[:], pb[:])

            # accumulators (written by the first combine of each o-half)
            atiles = [apool.tile([128, O], f32, name=f"acc{j}", tag="acc")
                      for j in range(NB)]

            def f8_group(b, osl, w8tile, cidx, first):
                ps = ppool.tile([128, 512], f32, tag="ps")
                for j in range(NH // 2):
                    nc.tensor.matmul(
                        ps[:],
                        lhsT=st8[b][:, 2 * j:2 * j + 2, :],
                        rhs=w8tile[:, 2 * j:2 * j + 2, :],
                        start=(j == 0),
                        stop=(j == NH // 2 - 1),
                        perf_mode=DR,
                    )
                if first:
                    # overwrite: acc = c * ps  (no dependency on prior acc)
                    nc.vector.tensor_scalar(
                        atiles[b][:, osl], ps[:], ctiles[b][:, cidx:cidx + 1],
                        None, Alu.mult)
                else:
                    nc.vector.scalar_tensor_tensor(
                        atiles[b][:, osl], ps[:], ctiles[b][:, cidx:cidx + 1],
                        atiles[b][:, osl], Alu.mult, Alu.add)

            def bf16_group(b, osl, pieces, cidx, first=False):
                # pieces: list of (tile, h-slice-within-tile)
                ps = ppool.tile([128, 512], f32, tag="ps")
                n = 0
                for t, hs in pieces:
                    for h in hs:
                        nc.tensor.matmul(
                            ps[:],
                            lhsT=stb[b][:, n, :],
                            rhs=t[:, h, :],
                            start=(n == 0),
                            stop=(n == NH - 1),
                        )
                        n += 1
                if first:
                    nc.vector.tensor_scalar(
                        atiles[b][:, osl], ps[:], ctiles[b][:, cidx:cidx + 1],
                        None, Alu.mult)
                else:
                    nc.vector.scalar_tensor_tensor(
                        atiles[b][:, osl], ps[:], ctiles[b][:, cidx:cidx + 1],
                        atiles[b][:, osl], Alu.mult, Alu.add)

            drings = [nc.sync, nc.scalar, nc.gpsimd]
            ndrain = 0

            def finish(b, osl):
                # pb term last: acc += (1-mix) * prj_b, then drain
                nonlocal ndrain
                nc.vector.scalar_tensor_tensor(
                    atiles[b][:, osl], pb_t[:, osl], ctiles[b][:, M:M + 1],
                    atiles[b][:, osl], Alu.mult, Alu.add)
                drings[ndrain % 3].dma_start(
                    out_r[:, b, osl], atiles[b][:, osl])
                ndrain += 1

            # ---- o-half 0: f8 m0, f8 m1, bf16 m2, bf16 m3, f8 base ----
            osl0 = slice(0, 512)
            for b in range(NB):
                f8_group(b, osl0, m08_t[0][:], 0, first=True)
            for b in range(NB):
                f8_group(b, osl0, m18_t[0][:], 1, first=False)
            # o1 inputs stream during the long bf16 phases
            s0_o1 = wpool.tile([128, NH, 512], bf16, name="s0_o1", tag="w")
            nc.gpsimd.dma_start(s0_o1[:], wts[0][1])
            for b in range(NB):
                bf16_group(b, osl0, [(s0a, range(NH // 2)),
                                     (s0b, range(NH // 2))], 2)
            s1_o1 = wpool.tile([128, NH, 512], bf16, name="s1_o1", tag="w")
            nc.gpsimd.dma_start(s1_o1[:], wts[1][1])
            m08_t[1] = p8pool.tile([128, NH, 512], f8, name="m08_1", tag="p8")
            nc.gpsimd.dma_start(m08_t[1][:], m08[1])
            for b in range(NB):
                bf16_group(b, osl0, [(s1, range(NH))], 3)
            m18_t[1] = p8pool.tile([128, NH, 512], f8, name="m18_1", tag="p8")
            nc.gpsimd.dma_start(m18_t[1][:], m18[1])
            prj8_t[1] = p8pool.tile([128, NH, 512], f8, name="prj8_1", tag="p8")
            nc.gpsimd.dma_start(prj8_t[1][:], prj8[1])
            for b in range(NB):
                f8_group(b, osl0, prj8_t[0][:], M + 1, first=False)
                finish(b, osl0)
            # ---- o-half 1: b-outer, drains spread ----
            osl1 = slice(512, 1024)
            for b in range(NB):
                bf16_group(b, osl1, [(s0_o1, range(NH))], 2, first=True)
                bf16_group(b, osl1, [(s1_o1, range(NH))], 3)
                f8_group(b, osl1, m08_t[1][:], 0, first=False)
                f8_group(b, osl1, m18_t[1][:], 1, first=False)
                f8_group(b, osl1, prj8_t[1][:], M + 1, first=False)
                finish(b, osl1)

    nc.compile()
    return nc


def get_nc():
    global _cached_nc
    if _cached_nc is None:
        _cached_nc = _build_nc()
    return _cached_nc


def make_in_maps(state, spectral_entropy, curvature, modulation_basis,
                 gate_w, gate_b, prj_w, prj_b):
    gwb = np.zeros((128, 2 * M), np.float32)
    gwb[:, 0:M] = np.asarray(gate_w, np.float32).reshape(1, M)
    gwb[:, M:2 * M] = np.asarray(gate_b, np.float32).reshape(1, M)
    pb = np.ascontiguousarray(
        np.broadcast_to(np.asarray(prj_b, np.float32).reshape(1, O), (128, O)))

    # weights: [H, O] -> [o(NO), 128(h_in), h(NH), 512] big moving chunks
    def to_pieces(wmat):
        # [H, O] = [NH*128, NO*512] -> [NO, 128, NH, 512]
        return wmat.reshape(NH, 128, NO, 512).transpose(2, 1, 0, 3)

    wts = np.empty((M - 2, NO, 128, NH, 512), ml_dtypes.bfloat16)
    for j, k in enumerate((2, 3)):
        wts[j] = to_pieces(np.asarray(modulation_basis[k], np.float32)
                           ).astype(ml_dtypes.bfloat16)
    wts = np.ascontiguousarray(wts)

    # modes 0 and 1 have the smallest gate coefficients (E[c^2] 3-7x below
    # modes 2/3): fp8 e4m3 pieces, layout [o, 128(h_in), h_tile, 512]
    def to_f8_pieces(wmat):
        return np.ascontiguousarray(
            wmat.reshape(NH, 128, NO, 512).transpose(2, 1, 0, 3)
        ).astype(ml_dtypes.float8_e4m3)

    m08 = to_f8_pieces(np.asarray(modulation_basis[0], np.float32))
    m18 = to_f8_pieces(np.asarray(modulation_basis[1], np.float32))
    # prj_w sigma = 1/32: scale x32 into e4m3's normal range (the combine
    # coefficient carries the 1/32); layout [o, 128(h_in), h_tile, 512]
    prj8 = np.ascontiguousarray(
        (np.asarray(prj_w, np.float32) * 32.0)
        .reshape(NH, 128, NO, 512).transpose(2, 1, 0, 3)
    ).astype(ml_dtypes.float8_e4m3)

    in_maps = []
    for c in range(NCORES):
        sl = slice(c * BL, (c + 1) * BL)
        shard = np.asarray(state[sl], np.float32).reshape(NB, 128, NH, 128)
        stT = np.ascontiguousarray(shard.transpose(0, 3, 2, 1))
        sc = np.empty((128, 2 * NB), np.float32)
        sc[:, 0:NB] = np.asarray(
            spectral_entropy[sl], np.float32).reshape(NB, 128).T
        sc[:, NB:2 * NB] = np.asarray(
            curvature[sl], np.float32).reshape(NB, 128).T
        in_maps.append({
            "stateT": stT.astype(ml_dtypes.bfloat16),
            "stateT8": stT.astype(ml_dtypes.float8_e4m3),
            "wts": wts, "prj8": prj8, "m08": m08, "m18": m18,
            "sc": sc, "gwb": gwb, "pb": pb})
    return in_maps


def _install_ntff_hook():
    """Register the axon NTFF profiling hook if the image's antenv lacks it."""
    import sys, types
    if 'antenv.axon_hooks' in sys.modules:
        return
    mod = types.ModuleType('antenv.axon_hooks')
    mod._hook = None
    mod.set_axon_ntff_profile_hook = lambda h: setattr(mod, '_hook', h)
    mod.get_axon_ntff_profile_hook = lambda: mod._hook
    sys.modules['antenv.axon_hooks'] = mod
    import antenv
    antenv.axon_hooks = mod
    try:
        from trn_agent_boot.trn_boot import _ntff_profile_via_ctypes
        mod._hook = _ntff_profile_via_ctypes('/opt/axon/libaxon_pjrt.so')
    except Exception:
        pass


def kernel(state, spectral_entropy, curvature, modulation_basis,
           gate_w, gate_b, prj_w, prj_b):
    global LAST_EXEC_TIME_NS, LAST_TRACE
    from concourse import bass_utils

    state = np.asarray(state, np.float32)
    spectral_entropy = np.asarray(spectral_entropy, np.float32)
    curvature = np.asarray(curvature, np.float32)
    modulation_basis = np.asarray(modulation_basis, np.float32)
    gate_w = np.asarray(gate_w, np.float32)
    gate_b = np.asarray(gate_b, np.float32)
    prj_w = np.asarray(prj_w, np.float32)
    prj_b = np.asarray(prj_b, np.float32)

    nc = get_nc()
    in_maps = make_in_maps(state, spectral_entropy, curvature,
                           modulation_basis, gate_w, gate_b, prj_w, prj_b)

    trace = bool(int(os.environ.get("KERNEL_TRACE", "0")))
    kwargs = {}
    if trace:
        _install_ntff_hook()
        kwargs["trace"] = True

    res = bass_utils.run_bass_kernel_spmd(
        nc, in_maps, core_ids=list(range(NCORES)), **kwargs)
    LAST_EXEC_TIME_NS = res.exec_time_ns
    it = res.instructions_and_trace
    LAST_TRACE = it[1] if it else None
    return np.concatenate(
        [res.results[c]["out"] for c in range(NCORES)], axis=0)


# revision 19
# speedup vs baseline: 1.0027x; 1.0027x over previous
"""Trainium2 Bass kernel for nn_AutoeclecticResponderHead.

Math (per row b):
    w      = softmax(se_b * gate_w + gate_b)          # [4]
    mix    = sigmoid(curv_b)
    out_b  = (1-mix) * (state_b @ prj_w + prj_b) + mix * sum_m w_m * (state_b @ W_m)
           = sum_{k=0..4} c_k[b] * (state_b @ A_k)  +  c_4[b] * prj_b
    with A_0..3 = modulation_basis modes (c_k = mix*w_k), A_4 = prj_w (c_4 = 1-mix).

Sharding: data-parallel over batch, 1024 rows per core, weights replicated.

Per-core kernel (v5):
  - Host pre-casts state + weights to bf16 and pre-transposes layouts, so the
    device does no dtype conversion and input DMA bytes are halved vs fp32.
  - 640 bf16 matmuls: stationary state tile [128,128], moving weight piece
    [128,512] (dedicated contiguous tiles - a strided slice of a larger tile
    measurably slows the PE, and contiguous pieces DMA at full line rate).
  - Weight chunks are streamed just-in-time: chunk (o,k+1) DMAs are emitted
    between chunk (o,k)'s matmul groups and throttled by weight-pool buffer
    reuse, which pins the scheduler to the consumption order (an up-front
    DMA flood gets reordered and starves early chunks).
  - Each (b, o-half) output block drains on the otherwise-idle sync ring
    right after its last combine, so the tail is one 256KB drain instead of
    the baseline's 11us bunched write-out.
"""

import os
import numpy as np
import ml_dtypes

B, H, O, M = 8192, 1024, 1024, 4
NCORES = 8
BL = B // NCORES          # rows per core
NB = BL // 128            # b tiles per core
NH = H // 128             # h (contraction) tiles
NO = O // 512             # output column tiles
NK = M + 1                # modes + base projection

_cached_nc = None
LAST_EXEC_TIME_NS = None
LAST_TRACE = None


def _build_nc():
    import concourse.bacc as bacc
    import concourse.tile as tile
    from concourse import mybir

    f32 = mybir.dt.float32
    bf16 = mybir.dt.bfloat16
    f8 = mybir.dt.float8e4
    DR = mybir.MatmulPerfMode.DoubleRow
    Alu = mybir.AluOpType
    Act = mybir.ActivationFunctionType
    AxX = mybir.AxisListType.X

    nc = bacc.Bacc("TRN2", target_bir_lowering=False, debug=False,
                   num_devices=NCORES)

    # lhsT layout: [b_tile, h_in(part), h_tile, row]
    stateT = nc.dram_tensor("stateT", [NB, 128, NH, 128], bf16,
                            kind="ExternalInput").ap()
    stateT8 = nc.dram_tensor("stateT8", [NB, 128, NH, 128], f8,
                             kind="ExternalInput").ap()
    # bf16 moving chunks (modes 2,3): [j, o, 128(part), h, 512]
    wts = nc.dram_tensor("wts", [M - 2, NO, 128, NH, 512], bf16,
                         kind="ExternalInput").ap()
    m08 = nc.dram_tensor("m08", [NO, 128, NH, 512], f8,
                         kind="ExternalInput").ap()
    m18 = nc.dram_tensor("m18", [NO, 128, NH, 512], f8,
                         kind="ExternalInput").ap()
    prj8 = nc.dram_tensor("prj8", [NO, 128, NH, 512], f8,
                          kind="ExternalInput").ap()
    sc = nc.dram_tensor("sc", [128, 2 * NB], f32, kind="ExternalInput").ap()
    gwb = nc.dram_tensor("gwb", [128, 2 * M], f32, kind="ExternalInput").ap()
    pb = nc.dram_tensor("pb", [128, O], f32, kind="ExternalInput").ap()
    out = nc.dram_tensor("out", [BL, O], f32, kind="ExternalOutput").ap()

    out_r = out.rearrange("(t p) o -> p t o", p=128)            # [128, NB, O]

    with tile.TileContext(nc) as tc:
        with (
            tc.tile_pool(name="big", bufs=1) as bigpool,
            tc.tile_pool(name="w", bufs=3) as wpool,
            tc.tile_pool(name="w0", bufs=1) as w0pool,
            tc.tile_pool(name="st", bufs=NB) as stpool,
            tc.tile_pool(name="st8", bufs=NB) as st8pool,
            tc.tile_pool(name="p8", bufs=3 * NO) as p8pool,
            tc.tile_pool(name="acc", bufs=NB) as apool,
            tc.tile_pool(name="g", bufs=NB) as gpool,
            tc.tile_pool(name="c", bufs=NB) as cpool,
            tc.tile_pool(name="ps", bufs=8, space="PSUM") as ppool,
        ):
            # PE warm-up first on the vector queue (no DMA dependency):
            # ramps the HAM clock during the initial DMA window.
            warm_in = bigpool.tile([128, 512], bf16, tag="warm")
            nc.vector.memset(warm_in[:], 0.0)
            warm_ps = ppool.tile([128, 512], f32, tag="ps")
            for i in range(10):
                nc.tensor.matmul(
                    warm_ps[:], lhsT=warm_in[:, 0:128], rhs=warm_in[:],
                    start=(i == 0), stop=(i == 9))

            # DMA plan. dma_start issue costs ~1.3us of queue time, so the
            # latency-critical queues carry few, big transfers, and the
            # scalar queue runs the gating ACT ops BEFORE its weight DMAs.
            #   gpsimd: sc, gwb (gating inputs), m18[0], s1 chunk, pb,
            #           o1 chunks + o1 f8 weights
            #   sync:   st8 x8 (f8 phases open the kernel), stb x8, prj8[0]
            #   scalar: m08[0], s0 chunk split in two halves
            sc_t = bigpool.tile([128, 2 * NB], f32, tag="sc")
            nc.gpsimd.dma_start(sc_t[:], sc[:])
            gwb_t = bigpool.tile([128, 2 * M], f32, tag="gwb")
            nc.gpsimd.dma_start(gwb_t[:], gwb[:])
            m18_t = [None] * NO
            m18_t[0] = p8pool.tile([128, NH, 512], f8, name="m18_0", tag="p8")
            nc.gpsimd.dma_start(m18_t[0][:], m18[0])

            st8 = []
            for b in range(NB):
                s8 = st8pool.tile([128, NH, 128], f8, tag="st8")
                nc.sync.dma_start(s8[:], stateT8[b])
                st8.append(s8)

            m08_t = [None] * NO
            m08_t[0] = p8pool.tile([128, NH, 512], f8, name="m08_0", tag="p8")
            nc.scalar.dma_start(m08_t[0][:], m08[0])
            # mode2 o0 chunk in two halves for an earlier first matmul
            s0a = w0pool.tile([128, NH // 2, 512], bf16, tag="w0a")
            nc.scalar.dma_start(s0a[:], wts[0][0][:, 0:NH // 2, :])
            s0b = w0pool.tile([128, NH // 2, 512], bf16, tag="w0b")
            nc.scalar.dma_start(s0b[:], wts[0][0][:, NH // 2:NH, :])

            # Gating (vector + scalar ACT, emitted before any further
            # dma_start lands on the scalar queue). ctile columns:
            # [0:M] = mix*softmax, [M] = (1-mix), [M+1] = (1-mix)/32.
            logits, nmxs, es, mixs, ctiles = [], [], [], [], []
            for j in range(NB):
                s = sc_t[:, j:j + 1]
                logit = gpool.tile([128, M], f32, tag="logit")
                nc.vector.scalar_tensor_tensor(
                    logit[:], gwb_t[:, 0:M], s, gwb_t[:, M:2 * M],
                    Alu.mult, Alu.add)
                logits.append(logit)
                nmx = gpool.tile([128, 1], f32, tag="nmx")
                nc.vector.tensor_reduce(
                    nmx[:], logit[:], axis=AxX, op=Alu.max, negate=True)
                nmxs.append(nmx)
            for j in range(NB):
                e = gpool.tile([128, M], f32, tag="e")
                nc.scalar.activation(e[:], logits[j][:], Act.Exp, bias=nmxs[j][:])
                es.append(e)
            for j in range(NB):
                mix = gpool.tile([128, 1], f32, tag="mix")
                nc.scalar.activation(
                    mix[:], sc_t[:, NB + j:NB + j + 1], Act.Sigmoid)
                mixs.append(mix)
            for j in range(NB):
                sm = gpool.tile([128, 1], f32, tag="sm")
                nc.vector.reduce_sum(sm[:], es[j][:], axis=AxX)
                rin = gpool.tile([128, 1], f32, tag="rin")
                nc.vector.reciprocal(rin[:], sm[:])
                c = cpool.tile([128, M + 2], f32, tag="c")
                nc.vector.tensor_scalar(
                    c[:, 0:M], es[j][:], rin[:], mixs[j][:], Alu.mult, Alu.mult)
                nc.vector.tensor_scalar(
                    c[:, M:M + 1], mixs[j][:], -1.0, 1.0, Alu.mult, Alu.add)
                nc.vector.tensor_scalar(
                    c[:, M + 1:M + 2], mixs[j][:], -1.0 / 32.0, 1.0 / 32.0,
                    Alu.mult, Alu.add)
                ctiles.append(c)

            # Remaining inputs, after the gating chain is unblocked.
            stb = []
            for b in range(NB):
                st = stpool.tile([128, NH, 128], bf16, tag="st")
                # last two tiles ride the scalar ring (free after its three
                # early loads) for arrival margin in the late bf16 groups
                ring = nc.sync if b < NB - 2 else nc.scalar
                ring.dma_start(st[:], stateT[b])
                stb.append(st)
            prj8_t = [None] * NO
            prj8_t[0] = p8pool.tile([128, NH, 512], f8, name="prj8_0", tag="p8")
            nc.sync.dma_start(prj8_t[0][:], prj8[0])

            s1 = wpool.tile([128, NH, 512], bf16, name="s1", tag="w")
            nc.gpsimd.dma_start(s1[:], wts[1][0])
            pb_t = bigpool.tile([128, O], f32, tag="pb")
            nc.gpsimd.dma_start(pb_t[:], pb[:])

            # accumulators (written by the first combine of each o-half)
            atiles = [apool.tile([128, O], f32, name=f"acc{j}", tag="acc")
                      for j in range(NB)]

            def f8_group(b, osl, w8tile, cidx, first):
                ps = ppool.tile([128, 512], f32, tag="ps")
                for j in range(NH // 2):
                    nc.tensor.matmul(
                        ps[:],
                        lhsT=st8[b][:, 2 * j:2 * j + 2, :],
                        rhs=w8tile[:, 2 * j:2 * j + 2, :],
                        start=(j == 0),
                        stop=(j == NH // 2 - 1),
                        perf_mode=DR,
                    )
                if first:
                    # overwrite: acc = c * ps  (no dependency on prior acc)
                    nc.vector.tensor_scalar(
                        atiles[b][:, osl], ps[:], ctiles[b][:, cidx:cidx + 1],
                        None, Alu.mult)
                else:
                    nc.vector.scalar_tensor_tensor(
                        atiles[b][:, osl], ps[:], ctiles[b][:, cidx:cidx + 1],
                        atiles[b][:, osl], Alu.mult, Alu.add)

            def bf16_group(b, osl, pieces, cidx, first=False):
                # pieces: list of (tile, h-slice-within-tile)
                ps = ppool.tile([128, 512], f32, tag="ps")
                n = 0
                for t, hs in pieces:
                    for h in hs:
                        nc.tensor.matmul(
                            ps[:],
                            lhsT=stb[b][:, n, :],
                            rhs=t[:, h, :],
                            start=(n == 0),
                            stop=(n == NH - 1),
                        )
                        n += 1
                if first:
                    nc.vector.tensor_scalar(
                        atiles[b][:, osl], ps[:], ctiles[b][:, cidx:cidx + 1],
                        None, Alu.mult)
                else:
                    nc.vector.scalar_tensor_tensor(
                        atiles[b][:, osl], ps[:], ctiles[b][:, cidx:cidx + 1],
                        atiles[b][:, osl], Alu.mult, Alu.add)

            drings = [nc.sync, nc.scalar, nc.gpsimd]
            ndrain = 0

            def finish(b, osl):
                # pb term last: acc += (1-mix) * prj_b, then drain
                nonlocal ndrain
                nc.vector.scalar_tensor_tensor(
                    atiles[b][:, osl], pb_t[:, osl], ctiles[b][:, M:M + 1],
                    atiles[b][:, osl], Alu.mult, Alu.add)
                drings[ndrain % 3].dma_start(
                    out_r[:, b, osl], atiles[b][:, osl])
                ndrain += 1

            # ---- o-half 0: f8 m0, f8 m1, bf16 m2, bf16 m3, f8 base ----
            osl0 = slice(0, 512)
            for b in range(NB):
                f8_group(b, osl0, m08_t[0][:], 0, first=True)
            # o1 inputs stream during the long bf16 phases
            s0_o1 = wpool.tile([128, NH, 512], bf16, name="s0_o1", tag="w")
            nc.gpsimd.dma_start(s0_o1[:], wts[0][1])
            for b in range(NB):
                bf16_group(b, osl0, [(s0a, range(NH // 2)),
                                     (s0b, range(NH // 2))], 2)
            for b in range(NB):
                f8_group(b, osl0, m18_t[0][:], 1, first=False)
            s1_o1 = wpool.tile([128, NH, 512], bf16, name="s1_o1", tag="w")
            nc.gpsimd.dma_start(s1_o1[:], wts[1][1])
            m08_t[1] = p8pool.tile([128, NH, 512], f8, name="m08_1", tag="p8")
            nc.gpsimd.dma_start(m08_t[1][:], m08[1])
            for b in range(NB):
                bf16_group(b, osl0, [(s1, range(NH))], 3)
            m18_t[1] = p8pool.tile([128, NH, 512], f8, name="m18_1", tag="p8")
            nc.gpsimd.dma_start(m18_t[1][:], m18[1])
            prj8_t[1] = p8pool.tile([128, NH, 512], f8, name="prj8_1", tag="p8")
            nc.gpsimd.dma_start(prj8_t[1][:], prj8[1])
            for b in range(NB):
                f8_group(b, osl0, prj8_t[0][:], M + 1, first=False)
                finish(b, osl0)
            # ---- o-half 1: b-outer, drains spread ----
            osl1 = slice(512, 1024)
            for b in range(NB):
                bf16_group(b, osl1, [(s0_o1, range(NH))], 2, first=True)
                bf16_group(b, osl1, [(s1_o1, range(NH))], 3)
                f8_group(b, osl1, m08_t[1][:], 0, first=False)
                f8_group(b, osl1, m18_t[1][:], 1, first=False)
                f8_group(b, osl1, prj8_t[1][:], M + 1, first=False)
                finish(b, osl1)

    nc.compile()
    return nc


def get_nc():
    global _cached_nc
    if _cached_nc is None:
        _cached_nc = _build_nc()
    return _cached_nc


def make_in_maps(state, spectral_entropy, curvature, modulation_basis,
                 gate_w, gate_b, prj_w, prj_b):
    gwb = np.zeros((128, 2 * M), np.float32)
    gwb[:, 0:M] = np.asarray(gate_w, np.float32).reshape(1, M)
    gwb[:, M:2 * M] = np.asarray(gate_b, np.float32).reshape(1, M)
    pb = np.ascontiguousarray(
        np.broadcast_to(np.asarray(prj_b, np.float32).reshape(1, O), (128, O)))

    # weights: [H, O] -> [o(NO), 128(h_in), h(NH), 512] big moving chunks
    def to_pieces(wmat):
        # [H, O] = [NH*128, NO*512] -> [NO, 128, NH, 512]
        return wmat.reshape(NH, 128, NO, 512).transpose(2, 1, 0, 3)

    wts = np.empty((M - 2, NO, 128, NH, 512), ml_dtypes.bfloat16)
    for j, k in enumerate((2, 3)):
        wts[j] = to_pieces(np.asarray(modulation_basis[k], np.float32)
                           ).astype(ml_dtypes.bfloat16)
    wts = np.ascontiguousarray(wts)

    # modes 0 and 1 have the smallest gate coefficients (E[c^2] 3-7x below
    # modes 2/3): fp8 e4m3 pieces, layout [o, 128(h_in), h_tile, 512]
    def to_f8_pieces(wmat):
        return np.ascontiguousarray(
            wmat.reshape(NH, 128, NO, 512).transpose(2, 1, 0, 3)
        ).astype(ml_dtypes.float8_e4m3)

    m08 = to_f8_pieces(np.asarray(modulation_basis[0], np.float32))
    m18 = to_f8_pieces(np.asarray(modulation_basis[1], np.float32))
    # prj_w sigma = 1/32: scale x32 into e4m3's normal range (the combine
    # coefficient carries the 1/32); layout [o, 128(h_in), h_tile, 512]
    prj8 = np.ascontiguousarray(
        (np.asarray(prj_w, np.float32) * 32.0)
        .reshape(NH, 128, NO, 512).transpose(2, 1, 0, 3)
    ).astype(ml_dtypes.float8_e4m3)

    in_maps = []
    for c in range(NCORES):
        sl = slice(c * BL, (c + 1) * BL)
        shard = np.asarray(state[sl], np.float32).reshape(NB, 128, NH, 128)
        stT = np.ascontiguousarray(shard.transpose(0, 3, 2, 1))
        sc = np.empty((128, 2 * NB), np.float32)
        sc[:, 0:NB] = np.asarray(
            spectral_entropy[sl], np.float32).reshape(NB, 128).T
        sc[:, NB:2 * NB] = np.asarray(
            curvature[sl], np.float32).reshape(NB, 128).T
        in_maps.append({
            "stateT": stT.astype(ml_dtypes.bfloat16),
            "stateT8": stT.astype(ml_dtypes.float8_e4m3),
            "wts": wts, "prj8": prj8, "m08": m08, "m18": m18,
            "sc": sc, "gwb": gwb, "pb": pb})
    return in_maps


def _install_ntff_hook():
    """Register the axon NTFF profiling hook if the image's antenv lacks it."""
    import sys, types
    if 'antenv.axon_hooks' in sys.modules:
        return
    mod = types.ModuleType('antenv.axon_hooks')
    mod._hook = None
    mod.set_axon_ntff_profile_hook = lambda h: setattr(mod, '_hook', h)
    mod.get_axon_ntff_profile_hook = lambda: mod._hook
    sys.modules['antenv.axon_hooks'] = mod
    import antenv
    antenv.axon_hooks = mod
    try:
        from trn_agent_boot.trn_boot import _ntff_profile_via_ctypes
        mod._hook = _ntff_profile_via_ctypes('/opt/axon/libaxon_pjrt.so')
    except Exception:
        pass


def kernel(state, spectral_entropy, curvature, modulation_basis,
           gate_w, gate_b, prj_w, prj_b):
    global LAST_EXEC_TIME_NS, LAST_TRACE
    from concourse import bass_utils

    state = np.asarray(state, np.float32)
    spectral_entropy = np.asarray(spectral_entropy, np.float32)
    curvature = np.asarray(curvature, np.float32)
    modulation_basis = np.asarray(modulation_basis, np.float32)
    gate_w = np.asarray(gate_w, np.float32)
    gate_b = np.asarray(gate_b, np.float32)
    prj_w = np.asarray(prj_w, np.float32)
    prj_b = np.asarray(prj_b, np.float32)

    nc = get_nc()
    in_maps = make_in_maps(state, spectral_entropy, curvature,
                           modulation_basis, gate_w, gate_b, prj_w, prj_b)

    trace = bool(int(os.environ.get("KERNEL_TRACE", "0")))
    kwargs = {}
    if trace:
        _install_ntff_hook()
        kwargs["trace"] = True

    res = bass_utils.run_bass_kernel_spmd(
        nc, in_maps, core_ids=list(range(NCORES)), **kwargs)
    LAST_EXEC_TIME_NS = res.exec_time_ns
    it = res.instructions_and_trace
    LAST_TRACE = it[1] if it else None
    return np.concatenate(
        [res.results[c]["out"] for c in range(NCORES)], axis=0)


# revision 20
# speedup vs baseline: 1.0095x; 1.0068x over previous
"""Trainium2 Bass kernel for nn_AutoeclecticResponderHead.

Math (per row b):
    w      = softmax(se_b * gate_w + gate_b)          # [4]
    mix    = sigmoid(curv_b)
    out_b  = (1-mix) * (state_b @ prj_w + prj_b) + mix * sum_m w_m * (state_b @ W_m)
           = sum_{k=0..4} c_k[b] * (state_b @ A_k)  +  c_4[b] * prj_b
    with A_0..3 = modulation_basis modes (c_k = mix*w_k), A_4 = prj_w (c_4 = 1-mix).

Sharding: data-parallel over batch, 1024 rows per core, weights replicated.

Per-core kernel (v5):
  - Host pre-casts state + weights to bf16 and pre-transposes layouts, so the
    device does no dtype conversion and input DMA bytes are halved vs fp32.
  - 640 bf16 matmuls: stationary state tile [128,128], moving weight piece
    [128,512] (dedicated contiguous tiles - a strided slice of a larger tile
    measurably slows the PE, and contiguous pieces DMA at full line rate).
  - Weight chunks are streamed just-in-time: chunk (o,k+1) DMAs are emitted
    between chunk (o,k)'s matmul groups and throttled by weight-pool buffer
    reuse, which pins the scheduler to the consumption order (an up-front
    DMA flood gets reordered and starves early chunks).
  - Each (b, o-half) output block drains on the otherwise-idle sync ring
    right after its last combine, so the tail is one 256KB drain instead of
    the baseline's 11us bunched write-out.
"""

import os
import numpy as np
import ml_dtypes

B, H, O, M = 8192, 1024, 1024, 4
NCORES = 8
BL = B // NCORES          # rows per core
NB = BL // 128            # b tiles per core
NH = H // 128             # h (contraction) tiles
NO = O // 512             # output column tiles
NK = M + 1                # modes + base projection

_cached_nc = None
LAST_EXEC_TIME_NS = None
LAST_TRACE = None


def _build_nc():
    import concourse.bacc as bacc
    import concourse.tile as tile
    from concourse import mybir

    f32 = mybir.dt.float32
    bf16 = mybir.dt.bfloat16
    f8 = mybir.dt.float8e4
    DR = mybir.MatmulPerfMode.DoubleRow
    Alu = mybir.AluOpType
    Act = mybir.ActivationFunctionType
    AxX = mybir.AxisListType.X

    nc = bacc.Bacc("TRN2", target_bir_lowering=False, debug=False,
                   num_devices=NCORES)

    # lhsT layout: [b_tile, h_in(part), h_tile, row]
    stateT = nc.dram_tensor("stateT", [NB, 128, NH, 128], bf16,
                            kind="ExternalInput").ap()
    stateT8 = nc.dram_tensor("stateT8", [NB, 128, NH, 128], f8,
                             kind="ExternalInput").ap()
    # bf16 moving chunks (modes 2,3): [j, o, 128(part), h, 512]
    wts = nc.dram_tensor("wts", [M - 2, NO, 128, NH, 512], bf16,
                         kind="ExternalInput").ap()
    m08 = nc.dram_tensor("m08", [NO, 128, NH, 512], f8,
                         kind="ExternalInput").ap()
    m18 = nc.dram_tensor("m18", [NO, 128, NH, 512], f8,
                         kind="ExternalInput").ap()
    prj8 = nc.dram_tensor("prj8", [NO, 128, NH, 512], f8,
                          kind="ExternalInput").ap()
    sc = nc.dram_tensor("sc", [128, 2 * NB], f32, kind="ExternalInput").ap()
    gwb = nc.dram_tensor("gwb", [128, 2 * M], f32, kind="ExternalInput").ap()
    pb = nc.dram_tensor("pb", [128, O], f32, kind="ExternalInput").ap()
    out = nc.dram_tensor("out", [BL, O], f32, kind="ExternalOutput").ap()

    out_r = out.rearrange("(t p) o -> p t o", p=128)            # [128, NB, O]

    with tile.TileContext(nc) as tc:
        with (
            tc.tile_pool(name="big", bufs=1) as bigpool,
            tc.tile_pool(name="w", bufs=3) as wpool,
            tc.tile_pool(name="w0", bufs=1) as w0pool,
            tc.tile_pool(name="st", bufs=NB) as stpool,
            tc.tile_pool(name="st8", bufs=NB) as st8pool,
            tc.tile_pool(name="p8", bufs=3 * NO) as p8pool,
            tc.tile_pool(name="acc", bufs=NB) as apool,
            tc.tile_pool(name="g", bufs=NB) as gpool,
            tc.tile_pool(name="c", bufs=NB) as cpool,
            tc.tile_pool(name="ps", bufs=8, space="PSUM") as ppool,
        ):
            # PE warm-up first on the vector queue (no DMA dependency):
            # ramps the HAM clock during the initial DMA window.
            warm_in = bigpool.tile([128, 512], bf16, tag="warm")
            nc.vector.memset(warm_in[:], 0.0)
            warm_ps = ppool.tile([128, 512], f32, tag="ps")
            for i in range(10):
                nc.tensor.matmul(
                    warm_ps[:], lhsT=warm_in[:, 0:128], rhs=warm_in[:],
                    start=(i == 0), stop=(i == 9))

            # DMA plan. dma_start issue costs ~1.3us of queue time, so the
            # latency-critical queues carry few, big transfers, and the
            # scalar queue runs the gating ACT ops BEFORE its weight DMAs.
            #   gpsimd: sc, gwb (gating inputs), m18[0], s1 chunk, pb,
            #           o1 chunks + o1 f8 weights
            #   sync:   st8 x8 (f8 phases open the kernel), stb x8, prj8[0]
            #   scalar: m08[0], s0 chunk split in two halves
            sc_t = bigpool.tile([128, 2 * NB], f32, tag="sc")
            nc.gpsimd.dma_start(sc_t[:], sc[:])
            gwb_t = bigpool.tile([128, 2 * M], f32, tag="gwb")
            nc.gpsimd.dma_start(gwb_t[:], gwb[:])
            m18_t = [None] * NO
            m18_t[0] = p8pool.tile([128, NH, 512], f8, name="m18_0", tag="p8")
            nc.gpsimd.dma_start(m18_t[0][:], m18[0])

            st8 = []
            for b in range(NB):
                s8 = st8pool.tile([128, NH, 128], f8, tag="st8")
                nc.sync.dma_start(s8[:], stateT8[b])
                st8.append(s8)

            m08_t = [None] * NO
            m08_t[0] = p8pool.tile([128, NH, 512], f8, name="m08_0", tag="p8")
            nc.scalar.dma_start(m08_t[0][:], m08[0])
            # mode2 o0 chunk in two halves for an earlier first matmul
            s0a = w0pool.tile([128, NH // 2, 512], bf16, tag="w0a")
            nc.scalar.dma_start(s0a[:], wts[0][0][:, 0:NH // 2, :])
            s0b = w0pool.tile([128, NH // 2, 512], bf16, tag="w0b")
            nc.scalar.dma_start(s0b[:], wts[0][0][:, NH // 2:NH, :])

            # Gating (vector + scalar ACT, emitted before any further
            # dma_start lands on the scalar queue). ctile columns:
            # [0:M] = mix*softmax, [M] = (1-mix), [M+1] = (1-mix)/32.
            logits, nmxs, es, mixs, ctiles = [], [], [], [], []
            for j in range(NB):
                s = sc_t[:, j:j + 1]
                logit = gpool.tile([128, M], f32, tag="logit")
                nc.vector.scalar_tensor_tensor(
                    logit[:], gwb_t[:, 0:M], s, gwb_t[:, M:2 * M],
                    Alu.mult, Alu.add)
                logits.append(logit)
                nmx = gpool.tile([128, 1], f32, tag="nmx")
                nc.vector.tensor_reduce(
                    nmx[:], logit[:], axis=AxX, op=Alu.max, negate=True)
                nmxs.append(nmx)
            for j in range(NB):
                e = gpool.tile([128, M], f32, tag="e")
                nc.scalar.activation(e[:], logits[j][:], Act.Exp, bias=nmxs[j][:])
                es.append(e)
            for j in range(NB):
                mix = gpool.tile([128, 1], f32, tag="mix")
                nc.scalar.activation(
                    mix[:], sc_t[:, NB + j:NB + j + 1], Act.Sigmoid)
                mixs.append(mix)
            for j in range(NB):
                sm = gpool.tile([128, 1], f32, tag="sm")
                nc.vector.reduce_sum(sm[:], es[j][:], axis=AxX)
                rin = gpool.tile([128, 1], f32, tag="rin")
                nc.vector.reciprocal(rin[:], sm[:])
                c = cpool.tile([128, M + 2], f32, tag="c")
                nc.vector.tensor_scalar(
                    c[:, 0:M], es[j][:], rin[:], mixs[j][:], Alu.mult, Alu.mult)
                nc.vector.tensor_scalar(
                    c[:, M:M + 1], mixs[j][:], -1.0, 1.0, Alu.mult, Alu.add)
                nc.vector.tensor_scalar(
                    c[:, M + 1:M + 2], mixs[j][:], -1.0 / 32.0, 1.0 / 32.0,
                    Alu.mult, Alu.add)
                ctiles.append(c)

            # Remaining inputs, after the gating chain is unblocked.
            # State tiles balanced across all three rings in consumption
            # order (sync ~95GB/s alone would pace the early phases).
            stb = [stpool.tile([128, NH, 128], bf16, name=f"stb{b}", tag="st")
                   for b in range(NB)]
            for b in range(3):
                nc.sync.dma_start(stb[b][:], stateT[b])
            prj8_t = [None] * NO
            prj8_t[0] = p8pool.tile([128, NH, 512], f8, name="prj8_0", tag="p8")
            nc.sync.dma_start(prj8_t[0][:], prj8[0])
            for b in range(3, 5):
                nc.scalar.dma_start(stb[b][:], stateT[b])

            s1 = wpool.tile([128, NH, 512], bf16, name="s1", tag="w")
            nc.gpsimd.dma_start(s1[:], wts[1][0])
            pb_t = bigpool.tile([128, O], f32, tag="pb")
            nc.gpsimd.dma_start(pb_t[:], pb[:])
            for b in range(5, NB):
                nc.gpsimd.dma_start(stb[b][:], stateT[b])

            # accumulators (written by the first combine of each o-half)
            atiles = [apool.tile([128, O], f32, name=f"acc{j}", tag="acc")
                      for j in range(NB)]

            def f8_group(b, osl, w8tile, cidx, first):
                ps = ppool.tile([128, 512], f32, tag="ps")
                for j in range(NH // 2):
                    nc.tensor.matmul(
                        ps[:],
                        lhsT=st8[b][:, 2 * j:2 * j + 2, :],
                        rhs=w8tile[:, 2 * j:2 * j + 2, :],
                        start=(j == 0),
                        stop=(j == NH // 2 - 1),
                        perf_mode=DR,
                    )
                if first:
                    # overwrite: acc = c * ps  (no dependency on prior acc)
                    nc.vector.tensor_scalar(
                        atiles[b][:, osl], ps[:], ctiles[b][:, cidx:cidx + 1],
                        None, Alu.mult)
                else:
                    nc.vector.scalar_tensor_tensor(
                        atiles[b][:, osl], ps[:], ctiles[b][:, cidx:cidx + 1],
                        atiles[b][:, osl], Alu.mult, Alu.add)

            def bf16_group(b, osl, pieces, cidx, first=False):
                # pieces: list of (tile, h-slice-within-tile)
                ps = ppool.tile([128, 512], f32, tag="ps")
                n = 0
                for t, hs in pieces:
                    for h in hs:
                        nc.tensor.matmul(
                            ps[:],
                            lhsT=stb[b][:, n, :],
                            rhs=t[:, h, :],
                            start=(n == 0),
                            stop=(n == NH - 1),
                        )
                        n += 1
                if first:
                    nc.vector.tensor_scalar(
                        atiles[b][:, osl], ps[:], ctiles[b][:, cidx:cidx + 1],
                        None, Alu.mult)
                else:
                    nc.vector.scalar_tensor_tensor(
                        atiles[b][:, osl], ps[:], ctiles[b][:, cidx:cidx + 1],
                        atiles[b][:, osl], Alu.mult, Alu.add)

            drings = [nc.sync, nc.scalar, nc.gpsimd]
            ndrain = 0

            def finish(b, osl):
                # pb term last: acc += (1-mix) * prj_b, then drain
                nonlocal ndrain
                nc.vector.scalar_tensor_tensor(
                    atiles[b][:, osl], pb_t[:, osl], ctiles[b][:, M:M + 1],
                    atiles[b][:, osl], Alu.mult, Alu.add)
                drings[ndrain % 3].dma_start(
                    out_r[:, b, osl], atiles[b][:, osl])
                ndrain += 1

            # ---- o-half 0: f8 m0, f8 m1, bf16 m2, bf16 m3, f8 base ----
            osl0 = slice(0, 512)
            for b in range(NB):
                f8_group(b, osl0, m08_t[0][:], 0, first=True)
            for b in range(NB):
                f8_group(b, osl0, m18_t[0][:], 1, first=False)
            # o1 inputs stream during the long bf16 phases
            s0_o1 = wpool.tile([128, NH, 512], bf16, name="s0_o1", tag="w")
            nc.gpsimd.dma_start(s0_o1[:], wts[0][1])
            for b in range(NB):
                bf16_group(b, osl0, [(s0a, range(NH // 2)),
                                     (s0b, range(NH // 2))], 2)
            s1_o1 = wpool.tile([128, NH, 512], bf16, name="s1_o1", tag="w")
            nc.gpsimd.dma_start(s1_o1[:], wts[1][1])
            m08_t[1] = p8pool.tile([128, NH, 512], f8, name="m08_1", tag="p8")
            nc.gpsimd.dma_start(m08_t[1][:], m08[1])
            for b in range(NB):
                bf16_group(b, osl0, [(s1, range(NH))], 3)
            m18_t[1] = p8pool.tile([128, NH, 512], f8, name="m18_1", tag="p8")
            nc.gpsimd.dma_start(m18_t[1][:], m18[1])
            prj8_t[1] = p8pool.tile([128, NH, 512], f8, name="prj8_1", tag="p8")
            nc.gpsimd.dma_start(prj8_t[1][:], prj8[1])
            for b in range(NB):
                f8_group(b, osl0, prj8_t[0][:], M + 1, first=False)
                finish(b, osl0)
            # ---- o-half 1: b-outer, drains spread ----
            osl1 = slice(512, 1024)
            for b in range(NB):
                bf16_group(b, osl1, [(s0_o1, range(NH))], 2, first=True)
                bf16_group(b, osl1, [(s1_o1, range(NH))], 3)
                f8_group(b, osl1, m08_t[1][:], 0, first=False)
                f8_group(b, osl1, m18_t[1][:], 1, first=False)
                f8_group(b, osl1, prj8_t[1][:], M + 1, first=False)
                finish(b, osl1)

    nc.compile()
    return nc


def get_nc():
    global _cached_nc
    if _cached_nc is None:
        _cached_nc = _build_nc()
    return _cached_nc


def make_in_maps(state, spectral_entropy, curvature, modulation_basis,
                 gate_w, gate_b, prj_w, prj_b):
    gwb = np.zeros((128, 2 * M), np.float32)
    gwb[:, 0:M] = np.asarray(gate_w, np.float32).reshape(1, M)
    gwb[:, M:2 * M] = np.asarray(gate_b, np.float32).reshape(1, M)
    pb = np.ascontiguousarray(
        np.broadcast_to(np.asarray(prj_b, np.float32).reshape(1, O), (128, O)))

    # weights: [H, O] -> [o(NO), 128(h_in), h(NH), 512] big moving chunks
    def to_pieces(wmat):
        # [H, O] = [NH*128, NO*512] -> [NO, 128, NH, 512]
        return wmat.reshape(NH, 128, NO, 512).transpose(2, 1, 0, 3)

    wts = np.empty((M - 2, NO, 128, NH, 512), ml_dtypes.bfloat16)
    for j, k in enumerate((2, 3)):
        wts[j] = to_pieces(np.asarray(modulation_basis[k], np.float32)
                           ).astype(ml_dtypes.bfloat16)
    wts = np.ascontiguousarray(wts)

    # modes 0 and 1 have the smallest gate coefficients (E[c^2] 3-7x below
    # modes 2/3): fp8 e4m3 pieces, layout [o, 128(h_in), h_tile, 512]
    def to_f8_pieces(wmat):
        return np.ascontiguousarray(
            wmat.reshape(NH, 128, NO, 512).transpose(2, 1, 0, 3)
        ).astype(ml_dtypes.float8_e4m3)

    m08 = to_f8_pieces(np.asarray(modulation_basis[0], np.float32))
    m18 = to_f8_pieces(np.asarray(modulation_basis[1], np.float32))
    # prj_w sigma = 1/32: scale x32 into e4m3's normal range (the combine
    # coefficient carries the 1/32); layout [o, 128(h_in), h_tile, 512]
    prj8 = np.ascontiguousarray(
        (np.asarray(prj_w, np.float32) * 32.0)
        .reshape(NH, 128, NO, 512).transpose(2, 1, 0, 3)
    ).astype(ml_dtypes.float8_e4m3)

    in_maps = []
    for c in range(NCORES):
        sl = slice(c * BL, (c + 1) * BL)
        shard = np.asarray(state[sl], np.float32).reshape(NB, 128, NH, 128)
        stT = np.ascontiguousarray(shard.transpose(0, 3, 2, 1))
        sc = np.empty((128, 2 * NB), np.float32)
        sc[:, 0:NB] = np.asarray(
            spectral_entropy[sl], np.float32).reshape(NB, 128).T
        sc[:, NB:2 * NB] = np.asarray(
            curvature[sl], np.float32).reshape(NB, 128).T
        in_maps.append({
            "stateT": stT.astype(ml_dtypes.bfloat16),
            "stateT8": stT.astype(ml_dtypes.float8_e4m3),
            "wts": wts, "prj8": prj8, "m08": m08, "m18": m18,
            "sc": sc, "gwb": gwb, "pb": pb})
    return in_maps


def _install_ntff_hook():
    """Register the axon NTFF profiling hook if the image's antenv lacks it."""
    import sys, types
    if 'antenv.axon_hooks' in sys.modules:
        return
    mod = types.ModuleType('antenv.axon_hooks')
    mod._hook = None
    mod.set_axon_ntff_profile_hook = lambda h: setattr(mod, '_hook', h)
    mod.get_axon_ntff_profile_hook = lambda: mod._hook
    sys.modules['antenv.axon_hooks'] = mod
    import antenv
    antenv.axon_hooks = mod
    try:
        from trn_agent_boot.trn_boot import _ntff_profile_via_ctypes
        mod._hook = _ntff_profile_via_ctypes('/opt/axon/libaxon_pjrt.so')
    except Exception:
        pass


def kernel(state, spectral_entropy, curvature, modulation_basis,
           gate_w, gate_b, prj_w, prj_b):
    global LAST_EXEC_TIME_NS, LAST_TRACE
    from concourse import bass_utils

    state = np.asarray(state, np.float32)
    spectral_entropy = np.asarray(spectral_entropy, np.float32)
    curvature = np.asarray(curvature, np.float32)
    modulation_basis = np.asarray(modulation_basis, np.float32)
    gate_w = np.asarray(gate_w, np.float32)
    gate_b = np.asarray(gate_b, np.float32)
    prj_w = np.asarray(prj_w, np.float32)
    prj_b = np.asarray(prj_b, np.float32)

    nc = get_nc()
    in_maps = make_in_maps(state, spectral_entropy, curvature,
                           modulation_basis, gate_w, gate_b, prj_w, prj_b)

    trace = bool(int(os.environ.get("KERNEL_TRACE", "0")))
    kwargs = {}
    if trace:
        _install_ntff_hook()
        kwargs["trace"] = True

    res = bass_utils.run_bass_kernel_spmd(
        nc, in_maps, core_ids=list(range(NCORES)), **kwargs)
    LAST_EXEC_TIME_NS = res.exec_time_ns
    it = res.instructions_and_trace
    LAST_TRACE = it[1] if it else None
    return np.concatenate(
        [res.results[c]["out"] for c in range(NCORES)], axis=0)
